# revision 53
# baseline (speedup 1.0000x reference)
"""Trainium2 kernel for nn_MessagePassing_22497038696556 (gnn_message_passing).

Strategy (edge-parallel over 8 NeuronCores, per the sharding hint):
  - Edges are split into 8 equal shards (in original order; the device
    work is purely per-edge so order is irrelevant on device).
  - The dominant FLOPs -- both per-edge MLPs
      w  = silu(es @ fc1_w1/4) @ fc1_w2/8   [E,32]
      w2 = silu(es @ fc2_w1/4) @ fc2_w2/8   [E,40]
    run on-device as one fused SPMD Bass/Tile kernel.  es is shipped
    fp16 in a [16, E] feature-on-partition layout; one K=16 matmul
    against the concatenated [16,128] W1, a single Silu op, and one
    K=128 matmul against a block-diagonal [128,72] W2 produce both
    layers' weights, returned fp16 as [32, E] + [40, E].

The axon tunnel (~45 MB/s) is the bottleneck, so the implementation
is organized around minimizing and hiding tunnel traffic:
  - fp16 wire format both directions (26 MB up, 117 MB down).
  - No donated pre-zeroed output buffers are uploaded (the kernel
    writes every output element, so they are simply omitted).
  - The device call is asynchronous; both outputs are fetched by
    background threads while the host runs the dst-argsort, gathers and
    layer-1 math.  The dst-sort permutation is applied to the fetched
    outputs inside those threads.
  - All one-time costs (bass build, Tile scheduling, NEFF compile/load,
    jit trace, device init) happen at import via a dummy-shape warmup.
  - On a device failure: one backend-reset retry, then a host fallback
    computes the same fp16-quantized MLPs on CPU.

Host side (float32, edge-last [feat, E] layout to avoid big transposes):
node-level linears, xf[src] / y0,y1[src] gathers, the per-edge tensor
products, and the segment-sum into nodes as a vectorized np.add.reduceat
over the dst-sorted edge order.
"""

import time
import numpy as np

N = 50000
E = 800000
NUM_NEIGHBORS = 16.0
S3 = 3.0 ** 0.5
N_CORES = 8
E_SHARD = E // N_CORES           # 100000
SUP = 25                         # supertiles per shard
SUP_W = 4096
E_PAD = SUP * SUP_W              # 102400
LAST_EXEC_NS = None
LAST_PATH = None                 # "device" | "device-retry" | "host-fallback"

_CACHED = {}


def _build_bass():
    import concourse.mybir as mybir
    import concourse.tile as tile
    from concourse import bacc

    f16 = mybir.dt.float16
    f32 = mybir.dt.float32
    nc = bacc.Bacc(None, target_bir_lowering=False)

    es_t = nc.dram_tensor("es_t", [16, E_PAD], f16, kind="ExternalInput")
    w1c = nc.dram_tensor("w1c", [16, 128], f16, kind="ExternalInput")
    w2bd = nc.dram_tensor("w2bd", [128, 72], f16, kind="ExternalInput")
    wout1 = nc.dram_tensor("wout1", [32, E_PAD], f16, kind="ExternalOutput")
    wout2 = nc.dram_tensor("wout2", [40, E_PAD], f16, kind="ExternalOutput")

    with tile.TileContext(nc) as tc:
        with (
            tc.tile_pool(name="wpool", bufs=1) as wpool,
            tc.tile_pool(name="espool", bufs=3) as espool,
            tc.tile_pool(name="hpool", bufs=3) as hpool,
            tc.tile_pool(name="opool", bufs=3) as opool,
            tc.tile_pool(name="ps1", bufs=4, space="PSUM") as ps1,
            tc.tile_pool(name="ps2", bufs=4, space="PSUM") as ps2,
        ):
            w1_t = wpool.tile([16, 128], f16, tag="w1")
            nc.sync.dma_start(out=w1_t[:], in_=w1c[:])
            w2_t = wpool.tile([128, 72], f16, tag="w2")
            nc.sync.dma_start(out=w2_t[:], in_=w2bd[:])

            for s in range(SUP):
                es_tile = espool.tile([16, SUP_W], f16, tag="es")
                nc.sync.dma_start(out=es_tile[:], in_=es_t[:, s * SUP_W:(s + 1) * SUP_W])
                o = opool.tile([72, SUP_W], f16, tag="o")
                for j in range(SUP_W // 512):
                    sl = slice(j * 512, (j + 1) * 512)
                    p1 = ps1.tile([128, 512], f32, tag="p1")
                    nc.tensor.matmul(p1[:], lhsT=w1_t[:], rhs=es_tile[:, sl],
                                     start=True, stop=True)
                    h = hpool.tile([128, 512], f16, tag="h")
                    nc.scalar.activation(h[:], p1[:],
                                         mybir.ActivationFunctionType.Silu)
                    p2 = ps2.tile([72, 512], f32, tag="p2")
                    nc.tensor.matmul(p2[:], lhsT=w2_t[:], rhs=h[:],
                                     start=True, stop=True)
                    nc.scalar.copy(o[:, sl], p2[:])
                nc.sync.dma_start(out=wout1[:, s * SUP_W:(s + 1) * SUP_W], in_=o[:32])
                nc.sync.dma_start(out=wout2[:, s * SUP_W:(s + 1) * SUP_W], in_=o[32:])
    nc.compile()
    return nc


def _make_spmd(nc):
    """Build the 8-core PJRT/shard_map callable for nc.
    Unlike concourse's run_bass_via_pjrt, no pre-zeroed output buffers are
    shipped host->device: this kernel writes every output element."""
    import jax
    from jax.sharding import Mesh, PartitionSpec
    from jax.experimental.shard_map import shard_map
    import concourse.mybir as mybir
    from concourse import bass2jax

    bass2jax.install_neuronx_cc_hook()

    partition_name = nc.partition_id_tensor.name if nc.partition_id_tensor else None
    in_names, out_names, out_avals = [], [], []
    for alloc in nc.m.functions[0].allocations:
        if not isinstance(alloc, mybir.MemoryLocationSet):
            continue
        name = alloc.memorylocations[0].name
        if alloc.kind == "ExternalInput":
            if name != partition_name:
                in_names.append(name)
        elif alloc.kind == "ExternalOutput":
            out_names.append(name)
            out_avals.append(jax.core.ShapedArray(
                tuple(alloc.tensor_shape), mybir.dt.np(alloc.dtype)))
    n_params = len(in_names)
    n_outs = len(out_names)
    # This kernel writes every element of its outputs, so the donated
    # pre-zeroed output buffers run_bass_via_pjrt ships host->device are
    # unnecessary: pass only the real inputs (the bass_exec parameter-order
    # check still holds) and let PJRT allocate the outputs on device.
    all_in_names = tuple(in_names)
    if partition_name is not None:
        all_in_names = all_in_names + (partition_name,)

    def _body(*args):
        operands = list(args)
        if partition_name is not None:
            operands.append(bass2jax.partition_id_tensor())
        outs = bass2jax._bass_exec_p.bind(
            *operands,
            out_avals=tuple(out_avals),
            in_names=all_in_names,
            out_names=tuple(out_names),
            lowering_input_output_aliases=(),
            sim_require_finite=True,
            sim_require_nnan=True,
            nc=nc,
        )
        return tuple(outs)

    devices = jax.devices()[:N_CORES]
    mesh = Mesh(np.asarray(devices), ("core",))
    spec = PartitionSpec("core")
    sharded = jax.jit(
        shard_map(_body, mesh=mesh, in_specs=(spec,) * n_params,
                  out_specs=(spec,) * n_outs, check_rep=False),
        keep_unused=True,
    )
    return sharded, tuple(in_names), tuple(out_names)


def _launch_spmd(concat_ins):
    """Launch the cached jit on 8 cores (async). concat_ins: dict name ->
    global array ([8*d0, ...]); returns dict name -> jax.Array (call
    np.asarray on them to block and fetch)."""
    if "jit" not in _CACHED:
        if "nc" not in _CACHED:
            _CACHED["nc"] = _build_bass()
        _CACHED["jit"] = _make_spmd(_CACHED["nc"])
    sharded, in_names, out_names = _CACHED["jit"]
    out_arrs = sharded(*[concat_ins[name] for name in in_names])
    return dict(zip(out_names, out_arrs))


def _put_es_sharded(es_f32):
    """Cast/pack es per core shard and upload each shard asynchronously so
    the h2d transfer overlaps the packing of later shards. Returns
    (sharded jax.Array [8*16, E_PAD] fp16, list of per-core numpy shards
    kept for the CPU fallback)."""
    ok = True
    try:
        import jax
        from jax.sharding import Mesh, PartitionSpec, NamedSharding
        devices = jax.devices()[:N_CORES]
        mesh = Mesh(np.asarray(devices), ("core",))
        sharding = NamedSharding(mesh, PartitionSpec("core"))
    except Exception:
        ok = False
    esT = es_f32.T                            # [16,E] view
    np_shards, bufs = [], []
    for k in range(N_CORES):
        shard = np.zeros((16, E_PAD), np.float16)
        shard[:, :E_SHARD] = esT[:, k * E_SHARD:(k + 1) * E_SHARD]
        np_shards.append(shard)
        if ok:
            try:
                bufs.append(jax.device_put(shard, devices[k]))
            except Exception:
                ok = False
    arr = None
    if ok:
        try:
            arr = jax.make_array_from_single_device_arrays(
                (N_CORES * 16, E_PAD), sharding, bufs)
        except Exception:
            arr = None
    return arr, np_shards


def _host_mlp_half(es_shards, w1cat, wbd_half, hid_slice):
    """CPU fallback computing one layer's edge-MLP like the device kernel."""
    cols = wbd_half.shape[1]
    out = np.empty((cols, E), np.float32)
    w1T = w1cat.astype(np.float32).T          # [128,16]
    w2T = wbd_half.astype(np.float32).T       # [cols,64]
    for k in range(N_CORES):
        esT = es_shards[k][:, :E_SHARD].astype(np.float32)
        h = (w1T @ esT)[hid_slice]            # [64,E_SHARD]
        h *= _sigmoid(h)
        out[:, k * E_SHARD:(k + 1) * E_SHARD] = w2T @ h.astype(np.float16).astype(np.float32)
    return out


class _DeviceRun:
    """Async device run of the fused edge-MLP kernel with per-output
    threaded fetch, one resynchronizing retry, and a host CPU fallback.
    es is shipped in ORIGINAL edge order (the device MLP is per-edge, so
    order is irrelevant); the dst-sort column permutation, supplied later
    via set_perm(), is applied to the outputs inside the fetch threads."""

    def __init__(self, es_f32, fc1_w1, fc1_w2, fc2_w1, fc2_w2):
        import threading
        global LAST_PATH
        LAST_PATH = None
        f16 = np.float16
        self.perm = None
        self.perm_ready = threading.Event()
        self._w1_raw_done = threading.Event()
        self.w1cat = np.concatenate([fc1_w1 / 4.0, fc2_w1 / 4.0], axis=1).astype(f16)
        w2bd = np.zeros((128, 72), np.float32)
        w2bd[:64, :32] = fc1_w2 / 8.0
        w2bd[64:, 32:] = fc2_w2 / 8.0
        self.w2bd = w2bd.astype(f16)
        try:
            es_dev, self.es_shards = _put_es_sharded(es_f32)
        except Exception:
            es_dev = None
            self.es_shards = None
        self.concat_ins = {
            "es_t": es_dev,
            "w1c": np.tile(self.w1cat, (N_CORES, 1)),
            "w2bd": np.tile(self.w2bd, (N_CORES, 1)),
        }
        self.t0 = time.perf_counter()
        self.results = {}
        self.failed = False
        self.retried = False
        try:
            if es_dev is None:
                raise RuntimeError("es upload failed")
            self.arrs = _launch_spmd(self.concat_ins)
        except Exception:
            self.failed = True
            self.arrs = None
        self.threads = {}
        if not self.failed:
            for name in ("wout1", "wout2"):
                th = threading.Thread(target=self._pull, args=(name,), daemon=True)
                th.start()
                self.threads[name] = th

    def set_perm(self, perm):
        self.perm = perm
        self.perm_ready.set()

    def _pull(self, name):
        try:
            # strict transfer priority: wout1 owns the tunnel first (layer-1
            # blocks on it); wout2's shards then download while wout1's
            # take/upcast and the layer-1 host math run
            if name == "wout2":
                self._w1_raw_done.wait(timeout=60.0)
            # one global asarray per output (pipelines shard transfers far
            # better than per-shard requests, which pay round-trip latency)
            try:
                raw = np.asarray(self.arrs[name])    # [8*cols, E_PAD] fp16
            finally:
                if name == "wout1":
                    self._w1_raw_done.set()
            cols = raw.shape[0] // N_CORES
            wg = raw.reshape(N_CORES, cols, E_PAD)
            cmp = np.empty((cols, E), np.float16)
            for k in range(N_CORES):
                cmp[:, k * E_SHARD:(k + 1) * E_SHARD] = wg[k][:, :E_SHARD]
            # perm-gather at fp16 (halves the GIL-held take), then upcast to
            # f32 in one vectorized GIL-releasing pass
            self.perm_ready.wait()
            self.results[name] = cmp.take(self.perm, axis=1).astype(np.float32)
        except Exception:
            self.failed = True
            self._w1_raw_done.set()

    def _finish(self, name):
        global LAST_EXEC_NS, LAST_PATH
        if LAST_PATH is None:
            LAST_PATH = "device"
        th = self.threads.get(name)
        if th is not None:
            th.join()
        if name not in self.results:
            LAST_PATH = "device-retry"
            # launch or fetch failed: one synchronous backend-reset retry,
            # then a host fallback (the axon-tunneled device occasionally
            # reports NRT_EXEC_UNIT_UNRECOVERABLE)
            if not self.retried:
                self.retried = True
                try:
                    import jax
                    _CACHED.pop("jit", None)
                    jax.extend.backend.clear_backends()
                    jax.clear_caches()
                    time.sleep(1.0)
                    # re-upload es as a plain numpy arg (old device buffers
                    # died with the backend)
                    self.concat_ins["es_t"] = np.concatenate(self.es_shards, axis=0)
                    self.arrs = _launch_spmd(self.concat_ins)
                    self.threads = {}
                    for nm in ("wout1", "wout2"):
                        if nm not in self.results:
                            self._pull(nm)
                except Exception:
                    pass
            if name not in self.results:
                LAST_PATH = "host-fallback"
                if name == "wout1":
                    raw = _host_mlp_half(
                        self.es_shards, self.w1cat, self.w2bd[:64, :32], slice(0, 64))
                else:
                    raw = _host_mlp_half(
                        self.es_shards, self.w1cat, self.w2bd[64:, 32:], slice(64, 128))
                self.perm_ready.wait()
                self.results[name] = raw.take(self.perm, axis=1)
        LAST_EXEC_NS = int((time.perf_counter() - self.t0) * 1e9)
        return self.results[name]

    def w1T(self):
        return self._finish("wout1")

    def w2T(self):
        return self._finish("wout2")


def _sigmoid(x):
    return np.where(x >= 0, 1.0 / (1.0 + np.exp(-x)),
                    np.exp(x) / (1.0 + np.exp(x))).astype(np.float32)


def kernel(node_features, node_attr, edge_attr, edge_scalars,
           sc1_w, lin1_w, fc1_w1, fc1_w2, lin2_w0, lin2_w1, lin3_w,
           sc2_w, lin1b_w0, lin1b_w1, fc2_w1, fc2_w2, lin2b_w, lin3b_w,
           edge_src, edge_dst):
    f = np.float32
    x = np.asarray(node_features, f)
    a = np.asarray(node_attr, f)
    ea = np.asarray(edge_attr, f)
    es = np.asarray(edge_scalars, f)
    src = np.asarray(edge_src).astype(np.int64)
    dst = np.asarray(edge_dst).astype(np.int64)
    n = x.shape[0]
    inv_nn = f(1.0 / np.sqrt(NUM_NEIGHBORS))

    # ---- device: launch both edge MLPs (async; host work overlaps).
    # es ships fp16 feature-on-partition in ORIGINAL edge order (per-core
    # shards upload asynchronously while later shards are packed); the sort
    # permutation is applied to the device outputs in the fetch threads.
    run = _DeviceRun(es, np.asarray(fc1_w1, f), np.asarray(fc1_w2, f),
                     np.asarray(fc2_w1, f), np.asarray(fc2_w2, f))

    # dst-sort once; all per-edge arrays live in sorted order, edge-last layout
    perm = np.argsort(dst, kind="stable")
    run.set_perm(perm)
    src_s, dst_s = src[perm], dst[perm]

    eaT = ea.T
    sh0 = eaT[0, perm]                       # [E]
    sh1T = np.ascontiguousarray(eaT[1:4][:, perm])  # [3,E]

    # segment boundaries for reduceat over sorted dst
    counts = np.bincount(dst_s, minlength=n)
    starts = np.zeros(n, np.int64)
    np.cumsum(counts[:-1], out=starts[1:])
    empty = counts == 0

    def segsumT(valsT):
        out = np.add.reduceat(valsT, starts, axis=1)
        out[:, empty] = 0.0
        return out

    # ---- layer 1 (host; pre-w part, filling the idle wait for w) ----
    sc = np.concatenate([(x @ np.asarray(sc1_w, f)) / 4.0 * a,
                         np.zeros((n, 24), f)], axis=1)
    xf = (x @ np.asarray(lin1_w, f)) / 4.0 * a
    xsT = xf.T[:, src_s]                     # [16,E]
    # w-independent products, computed while the device round-trip runs;
    # efT is zero-filled (not just allocated) so its page faults land in
    # this idle window rather than in the timed multiplies below
    P0 = xsT * sh0                           # [16,E]
    Pc = [xsT * sh1T[c] for c in range(3)]   # 3x [16,E]
    efT = np.zeros((64, E), f)

    wT = run.w1T()        # [32,E] layer-1 weights (blocks on device fetch)

    def _efmul(lo):       # ef rows from precomputed products (threaded)
        if lo == 0:
            np.multiply(wT[:16], P0, out=efT[:16])
            np.multiply(wT[16:32], Pc[0], out=efT[16:64:3])
        else:
            np.multiply(wT[16:32], Pc[1], out=efT[17:64:3])
            np.multiply(wT[16:32], Pc[2], out=efT[18:64:3])
    import threading
    _th = threading.Thread(target=_efmul, args=(1,))
    _th.start()
    _efmul(0)
    _th.join()
    midT = segsumT(efT) * inv_nn             # [64,N]
    mid0 = np.ascontiguousarray(midT[:16].T)  # [N,16]
    mid1 = np.ascontiguousarray(midT[16:].T).reshape(n, 16, 3)
    conv0 = (mid0 @ np.asarray(lin2_w0, f)) / 4.0 * a
    conv1 = np.einsum("nuc,uw->nwc", mid1, np.asarray(lin2_w1, f)) / 4.0 * a[:, :, None]
    conv = np.concatenate([conv0, conv1.reshape(n, 24)], axis=1)
    ang = 0.1 * (mid0 @ np.asarray(lin3_w, f)) / 4.0 * a
    mask = np.concatenate([np.ones(40, f), np.zeros(24, f)])
    sin = 1.0 - mask + np.sin(ang) * mask
    y = np.cos(ang) * sc + sin * conv
    sig = _sigmoid(y[:, :32])
    h0 = y[:, :32] * sig
    gates = _sigmoid(y[:, 32:40])
    h1 = y[:, 40:].reshape(n, 8, 3) * gates[:, :, None]

    # ---- layer 2 (host except w2) ----
    inv32, inv8, inv40 = f(1 / np.sqrt(32.0)), f(1 / np.sqrt(8.0)), f(1 / np.sqrt(40.0))
    sc2 = (h0 @ np.asarray(sc2_w, f)) * inv32 * a
    y0 = (h0 @ np.asarray(lin1b_w0, f)) * inv32 * a
    y1 = np.einsum("nuc,uw->nwc", h1, np.asarray(lin1b_w1, f)) * inv8 * a[:, :, None]
    xs0T = y0.T[:, src_s]                    # [32,E]
    xs1T = y1.reshape(n, 24).T[:, src_s]     # [24,E] rows u*3+c
    # precompute all w2-independent factors before blocking on the fetch
    xs0T *= sh0                              # xs0*sh0
    d = (xs1T.reshape(8, 3, E) * sh1T[None, :, :]).sum(axis=1)  # [8,E]
    d *= f(1.0 / S3)
    efbT = np.zeros((40, E), f)              # pre-touch pages before the join
    w2T = run.w2T()       # [40,E] layer-2 weights (blocks on device fetch)
    np.multiply(w2T[:32], xs0T, out=efbT[:32])
    np.multiply(w2T[32:], d, out=efbT[32:])
    mid2 = segsumT(efbT).T * inv_nn          # [N,40]
    conv2 = (mid2 @ np.asarray(lin2b_w, f)) * inv40 * a
    ang2 = 0.1 * (mid2 @ np.asarray(lin3b_w, f)) * inv40 * a
    return (np.cos(ang2) * sc2 + np.sin(ang2) * conv2).astype(np.float32)


def _warmup():
    """Initialize the device stack at import: bass build, jit trace, NEFF
    compile/load, device init and one dummy execution. Shapes (not values)
    key all caches, so kernel() later only pays transfers + execution."""
    try:
        es_dev, _ = _put_es_sharded(np.zeros((E, 16), np.float32))
        if es_dev is None:
            es_dev = np.zeros((N_CORES * 16, E_PAD), np.float16)
        ins = {
            "es_t": es_dev,
            "w1c": np.zeros((N_CORES * 16, 128), np.float16),
            "w2bd": np.zeros((N_CORES * 128, 72), np.float16),
        }
        arrs = _launch_spmd(ins)
        for v in arrs.values():
            np.asarray(v)
    except Exception:
        _CACHED.pop("jit", None)


_warmup()


# revision 54
# speedup vs baseline: 1.0836x; 1.0836x over previous
"""Trainium2 kernel for nn_MessagePassing_22497038696556 (gnn_message_passing).

Strategy (edge-parallel over 8 NeuronCores, per the sharding hint):
  - Edges are split into 8 equal shards (in original order; the device
    work is purely per-edge so order is irrelevant on device).
  - The dominant FLOPs -- both per-edge MLPs
      w  = silu(es @ fc1_w1/4) @ fc1_w2/8   [E,32]
      w2 = silu(es @ fc2_w1/4) @ fc2_w2/8   [E,40]
    run on-device as one fused SPMD Bass/Tile kernel.  es is shipped
    fp16 in a [16, E] feature-on-partition layout; one K=16 matmul
    against the concatenated [16,128] W1, a single Silu op, and one
    K=128 matmul against a block-diagonal [128,72] W2 produce both
    layers' weights, returned fp16 as [32, E] + [40, E].

The axon tunnel (~45 MB/s) is the bottleneck, so the implementation
is organized around minimizing and hiding tunnel traffic:
  - fp16 wire format both directions (26 MB up, 117 MB down).
  - No donated pre-zeroed output buffers are uploaded (the kernel
    writes every output element, so they are simply omitted).
  - The device call is asynchronous; both outputs are fetched by
    background threads while the host runs the dst-argsort, gathers and
    layer-1 math.  The dst-sort permutation is applied to the fetched
    outputs inside those threads.
  - All one-time costs (bass build, Tile scheduling, NEFF compile/load,
    jit trace, device init) happen at import via a dummy-shape warmup.
  - On a device failure: one backend-reset retry, then a host fallback
    computes the same fp16-quantized MLPs on CPU.

Host side (float32, edge-last [feat, E] layout to avoid big transposes):
node-level linears, xf[src] / y0,y1[src] gathers, the per-edge tensor
products, and the segment-sum into nodes as a vectorized np.add.reduceat
over the dst-sorted edge order.
"""

import time
import numpy as np

N = 50000
E = 800000
NUM_NEIGHBORS = 16.0
S3 = 3.0 ** 0.5
N_CORES = 8
E_SHARD = E // N_CORES           # 100000
SUP = 25                         # supertiles per shard
SUP_W = 4096
E_PAD = SUP * SUP_W              # 102400
LAST_EXEC_NS = None
LAST_PATH = None                 # "device" | "device-retry" | "host-fallback"

_CACHED = {}


def _build_bass():
    import concourse.mybir as mybir
    import concourse.tile as tile
    from concourse import bacc

    f16 = mybir.dt.float16
    f32 = mybir.dt.float32
    nc = bacc.Bacc(None, target_bir_lowering=False)

    es_t = nc.dram_tensor("es_t", [16, E_PAD], f16, kind="ExternalInput")
    w1c = nc.dram_tensor("w1c", [16, 128], f16, kind="ExternalInput")
    w2bd = nc.dram_tensor("w2bd", [128, 72], f16, kind="ExternalInput")
    wout1 = nc.dram_tensor("wout1", [32, E_PAD], f16, kind="ExternalOutput")
    wout2 = nc.dram_tensor("wout2", [40, E_PAD], f16, kind="ExternalOutput")

    with tile.TileContext(nc) as tc:
        with (
            tc.tile_pool(name="wpool", bufs=1) as wpool,
            tc.tile_pool(name="espool", bufs=3) as espool,
            tc.tile_pool(name="hpool", bufs=3) as hpool,
            tc.tile_pool(name="opool", bufs=3) as opool,
            tc.tile_pool(name="ps1", bufs=4, space="PSUM") as ps1,
            tc.tile_pool(name="ps2", bufs=4, space="PSUM") as ps2,
        ):
            w1_t = wpool.tile([16, 128], f16, tag="w1")
            nc.sync.dma_start(out=w1_t[:], in_=w1c[:])
            w2_t = wpool.tile([128, 72], f16, tag="w2")
            nc.sync.dma_start(out=w2_t[:], in_=w2bd[:])

            for s in range(SUP):
                es_tile = espool.tile([16, SUP_W], f16, tag="es")
                nc.sync.dma_start(out=es_tile[:], in_=es_t[:, s * SUP_W:(s + 1) * SUP_W])
                o = opool.tile([72, SUP_W], f16, tag="o")
                for j in range(SUP_W // 512):
                    sl = slice(j * 512, (j + 1) * 512)
                    p1 = ps1.tile([128, 512], f32, tag="p1")
                    nc.tensor.matmul(p1[:], lhsT=w1_t[:], rhs=es_tile[:, sl],
                                     start=True, stop=True)
                    h = hpool.tile([128, 512], f16, tag="h")
                    nc.scalar.activation(h[:], p1[:],
                                         mybir.ActivationFunctionType.Silu)
                    p2 = ps2.tile([72, 512], f32, tag="p2")
                    nc.tensor.matmul(p2[:], lhsT=w2_t[:], rhs=h[:],
                                     start=True, stop=True)
                    nc.scalar.copy(o[:, sl], p2[:])
                nc.sync.dma_start(out=wout1[:, s * SUP_W:(s + 1) * SUP_W], in_=o[:32])
                nc.sync.dma_start(out=wout2[:, s * SUP_W:(s + 1) * SUP_W], in_=o[32:])
    nc.compile()
    return nc


def _make_spmd(nc):
    """Build the 8-core PJRT/shard_map callable for nc.
    Unlike concourse's run_bass_via_pjrt, no pre-zeroed output buffers are
    shipped host->device: this kernel writes every output element."""
    import jax
    from jax.sharding import Mesh, PartitionSpec
    from jax.experimental.shard_map import shard_map
    import concourse.mybir as mybir
    from concourse import bass2jax

    bass2jax.install_neuronx_cc_hook()

    partition_name = nc.partition_id_tensor.name if nc.partition_id_tensor else None
    in_names, out_names, out_avals = [], [], []
    for alloc in nc.m.functions[0].allocations:
        if not isinstance(alloc, mybir.MemoryLocationSet):
            continue
        name = alloc.memorylocations[0].name
        if alloc.kind == "ExternalInput":
            if name != partition_name:
                in_names.append(name)
        elif alloc.kind == "ExternalOutput":
            out_names.append(name)
            out_avals.append(jax.core.ShapedArray(
                tuple(alloc.tensor_shape), mybir.dt.np(alloc.dtype)))
    n_params = len(in_names)
    n_outs = len(out_names)
    # This kernel writes every element of its outputs, so the donated
    # pre-zeroed output buffers run_bass_via_pjrt ships host->device are
    # unnecessary: pass only the real inputs (the bass_exec parameter-order
    # check still holds) and let PJRT allocate the outputs on device.
    all_in_names = tuple(in_names)
    if partition_name is not None:
        all_in_names = all_in_names + (partition_name,)

    def _body(*args):
        operands = list(args)
        if partition_name is not None:
            operands.append(bass2jax.partition_id_tensor())
        outs = bass2jax._bass_exec_p.bind(
            *operands,
            out_avals=tuple(out_avals),
            in_names=all_in_names,
            out_names=tuple(out_names),
            lowering_input_output_aliases=(),
            sim_require_finite=True,
            sim_require_nnan=True,
            nc=nc,
        )
        return tuple(outs)

    devices = jax.devices()[:N_CORES]
    mesh = Mesh(np.asarray(devices), ("core",))
    spec = PartitionSpec("core")
    sharded = jax.jit(
        shard_map(_body, mesh=mesh, in_specs=(spec,) * n_params,
                  out_specs=(spec,) * n_outs, check_rep=False),
        keep_unused=True,
    )
    return sharded, tuple(in_names), tuple(out_names)


def _launch_spmd(concat_ins):
    """Launch the cached jit on 8 cores (async). concat_ins: dict name ->
    global array ([8*d0, ...]); returns dict name -> jax.Array (call
    np.asarray on them to block and fetch)."""
    if "jit" not in _CACHED:
        if "nc" not in _CACHED:
            _CACHED["nc"] = _build_bass()
        _CACHED["jit"] = _make_spmd(_CACHED["nc"])
    sharded, in_names, out_names = _CACHED["jit"]
    out_arrs = sharded(*[concat_ins[name] for name in in_names])
    return dict(zip(out_names, out_arrs))


def _put_es_sharded(es_f32):
    """Cast/pack es per core shard and upload each shard asynchronously so
    the h2d transfer overlaps the packing of later shards. Returns
    (sharded jax.Array [8*16, E_PAD] fp16, list of per-core numpy shards
    kept for the CPU fallback)."""
    ok = True
    try:
        import jax
        from jax.sharding import Mesh, PartitionSpec, NamedSharding
        devices = jax.devices()[:N_CORES]
        mesh = Mesh(np.asarray(devices), ("core",))
        sharding = NamedSharding(mesh, PartitionSpec("core"))
    except Exception:
        ok = False
    esT = es_f32.T                            # [16,E] view
    np_shards, bufs = [], []
    for k in range(N_CORES):
        shard = np.zeros((16, E_PAD), np.float16)
        shard[:, :E_SHARD] = esT[:, k * E_SHARD:(k + 1) * E_SHARD]
        np_shards.append(shard)
        if ok:
            try:
                bufs.append(jax.device_put(shard, devices[k]))
            except Exception:
                ok = False
    arr = None
    if ok:
        try:
            arr = jax.make_array_from_single_device_arrays(
                (N_CORES * 16, E_PAD), sharding, bufs)
        except Exception:
            arr = None
    return arr, np_shards


def _host_mlp_half(es_shards, w1cat, wbd_half, hid_slice):
    """CPU fallback computing one layer's edge-MLP like the device kernel."""
    cols = wbd_half.shape[1]
    out = np.empty((cols, E), np.float32)
    w1T = w1cat.astype(np.float32).T          # [128,16]
    w2T = wbd_half.astype(np.float32).T       # [cols,64]
    for k in range(N_CORES):
        esT = es_shards[k][:, :E_SHARD].astype(np.float32)
        h = (w1T @ esT)[hid_slice]            # [64,E_SHARD]
        h *= _sigmoid(h)
        out[:, k * E_SHARD:(k + 1) * E_SHARD] = w2T @ h.astype(np.float16).astype(np.float32)
    return out


class _DeviceRun:
    """Async device run of the fused edge-MLP kernel with per-output
    threaded fetch, one resynchronizing retry, and a host CPU fallback.
    es is shipped in ORIGINAL edge order (the device MLP is per-edge, so
    order is irrelevant); the dst-sort column permutation, supplied later
    via set_perm(), is applied to the outputs inside the fetch threads."""

    def __init__(self, es_f32, fc1_w1, fc1_w2, fc2_w1, fc2_w2):
        import threading
        global LAST_PATH
        LAST_PATH = None
        f16 = np.float16
        self.perm = None
        self.perm_ready = threading.Event()
        self._w1_raw_done = threading.Event()
        self.w1cat = np.concatenate([fc1_w1 / 4.0, fc2_w1 / 4.0], axis=1).astype(f16)
        w2bd = np.zeros((128, 72), np.float32)
        w2bd[:64, :32] = fc1_w2 / 8.0
        w2bd[64:, 32:] = fc2_w2 / 8.0
        self.w2bd = w2bd.astype(f16)
        try:
            es_dev, self.es_shards = _put_es_sharded(es_f32)
        except Exception:
            es_dev = None
            self.es_shards = None
        self.concat_ins = {
            "es_t": es_dev,
            "w1c": np.tile(self.w1cat, (N_CORES, 1)),
            "w2bd": np.tile(self.w2bd, (N_CORES, 1)),
        }
        self.t0 = time.perf_counter()
        self.results = {}
        self.failed = False
        self.retried = False
        try:
            if es_dev is None:
                raise RuntimeError("es upload failed")
            self.arrs = _launch_spmd(self.concat_ins)
        except Exception:
            self.failed = True
            self.arrs = None
        self.threads = {}
        if not self.failed:
            for name in ("wout1", "wout2"):
                th = threading.Thread(target=self._pull, args=(name,), daemon=True)
                th.start()
                self.threads[name] = th

    def set_perm(self, perm):
        self.perm = perm
        self.perm_ready.set()

    def _pull(self, name):
        try:
            # strict transfer priority: wout1 owns the tunnel first (layer-1
            # blocks on it); wout2's shards then download while wout1's
            # take/upcast and the layer-1 host math run
            if name == "wout2":
                self._w1_raw_done.wait(timeout=60.0)
            # one global asarray per output (pipelines shard transfers far
            # better than per-shard requests, which pay round-trip latency)
            try:
                raw = np.asarray(self.arrs[name])    # [8*cols, E_PAD] fp16
            finally:
                if name == "wout1":
                    self._w1_raw_done.set()
            cols = raw.shape[0] // N_CORES
            wg = raw.reshape(N_CORES, cols, E_PAD)
            cmp = np.empty((cols, E), np.float16)
            for k in range(N_CORES):
                cmp[:, k * E_SHARD:(k + 1) * E_SHARD] = wg[k][:, :E_SHARD]
            # perm-gather at fp16 (halves the GIL-held take); stay fp16 —
            # the downstream multiplies upcast on the fly, cheaper than a
            # separate astype pass on the critical path
            self.perm_ready.wait()
            self.results[name] = cmp.take(self.perm, axis=1)
        except Exception:
            self.failed = True
            self._w1_raw_done.set()

    def _finish(self, name):
        global LAST_EXEC_NS, LAST_PATH
        if LAST_PATH is None:
            LAST_PATH = "device"
        th = self.threads.get(name)
        if th is not None:
            th.join()
        if name not in self.results:
            LAST_PATH = "device-retry"
            # launch or fetch failed: one synchronous backend-reset retry,
            # then a host fallback (the axon-tunneled device occasionally
            # reports NRT_EXEC_UNIT_UNRECOVERABLE)
            if not self.retried:
                self.retried = True
                try:
                    import jax
                    _CACHED.pop("jit", None)
                    jax.extend.backend.clear_backends()
                    jax.clear_caches()
                    time.sleep(1.0)
                    # re-upload es as a plain numpy arg (old device buffers
                    # died with the backend)
                    self.concat_ins["es_t"] = np.concatenate(self.es_shards, axis=0)
                    self.arrs = _launch_spmd(self.concat_ins)
                    self.threads = {}
                    for nm in ("wout1", "wout2"):
                        if nm not in self.results:
                            self._pull(nm)
                except Exception:
                    pass
            if name not in self.results:
                LAST_PATH = "host-fallback"
                if name == "wout1":
                    raw = _host_mlp_half(
                        self.es_shards, self.w1cat, self.w2bd[:64, :32], slice(0, 64))
                else:
                    raw = _host_mlp_half(
                        self.es_shards, self.w1cat, self.w2bd[64:, 32:], slice(64, 128))
                self.perm_ready.wait()
                self.results[name] = raw.take(self.perm, axis=1)
        LAST_EXEC_NS = int((time.perf_counter() - self.t0) * 1e9)
        return self.results[name]

    def w1T(self):
        return self._finish("wout1")

    def w2T(self):
        return self._finish("wout2")


def _sigmoid(x):
    return np.where(x >= 0, 1.0 / (1.0 + np.exp(-x)),
                    np.exp(x) / (1.0 + np.exp(x))).astype(np.float32)


def kernel(node_features, node_attr, edge_attr, edge_scalars,
           sc1_w, lin1_w, fc1_w1, fc1_w2, lin2_w0, lin2_w1, lin3_w,
           sc2_w, lin1b_w0, lin1b_w1, fc2_w1, fc2_w2, lin2b_w, lin3b_w,
           edge_src, edge_dst):
    f = np.float32
    x = np.asarray(node_features, f)
    a = np.asarray(node_attr, f)
    ea = np.asarray(edge_attr, f)
    es = np.asarray(edge_scalars, f)
    src = np.asarray(edge_src).astype(np.int64)
    dst = np.asarray(edge_dst).astype(np.int64)
    n = x.shape[0]
    inv_nn = f(1.0 / np.sqrt(NUM_NEIGHBORS))

    # ---- device: launch both edge MLPs (async; host work overlaps).
    # es ships fp16 feature-on-partition in ORIGINAL edge order (per-core
    # shards upload asynchronously while later shards are packed); the sort
    # permutation is applied to the device outputs in the fetch threads.
    run = _DeviceRun(es, np.asarray(fc1_w1, f), np.asarray(fc1_w2, f),
                     np.asarray(fc2_w1, f), np.asarray(fc2_w2, f))

    # dst-sort once; all per-edge arrays live in sorted order, edge-last layout
    perm = np.argsort(dst, kind="stable")
    run.set_perm(perm)
    src_s, dst_s = src[perm], dst[perm]

    eaT = ea.T
    sh0 = eaT[0, perm]                       # [E]
    sh1T = np.ascontiguousarray(eaT[1:4][:, perm])  # [3,E]

    # segment boundaries for reduceat over sorted dst
    counts = np.bincount(dst_s, minlength=n)
    starts = np.zeros(n, np.int64)
    np.cumsum(counts[:-1], out=starts[1:])
    empty = counts == 0

    def segsumT(valsT):
        out = np.add.reduceat(valsT, starts, axis=1)
        out[:, empty] = 0.0
        return out

    # ---- layer 1 (host; pre-w part, filling the idle wait for w) ----
    sc = np.concatenate([(x @ np.asarray(sc1_w, f)) / 4.0 * a,
                         np.zeros((n, 24), f)], axis=1)
    xf = (x @ np.asarray(lin1_w, f)) / 4.0 * a
    xsT = xf.T[:, src_s]                     # [16,E]
    # w-independent products, computed while the device round-trip runs;
    # efT is zero-filled (not just allocated) so its page faults land in
    # this idle window rather than in the timed multiplies below
    P0 = xsT * sh0                           # [16,E]
    Pc = [xsT * sh1T[c] for c in range(3)]   # 3x [16,E]
    efT = np.zeros((64, E), f)

    wT = run.w1T()        # [32,E] layer-1 weights (blocks on device fetch)

    def _efmul(lo):       # ef rows from precomputed products (threaded)
        if lo == 0:
            np.multiply(wT[:16], P0, out=efT[:16])
            np.multiply(wT[16:32], Pc[0], out=efT[16:64:3])
        else:
            np.multiply(wT[16:32], Pc[1], out=efT[17:64:3])
            np.multiply(wT[16:32], Pc[2], out=efT[18:64:3])
    import threading
    _th = threading.Thread(target=_efmul, args=(1,))
    _th.start()
    _efmul(0)
    _th.join()
    midT = segsumT(efT) * inv_nn             # [64,N]
    mid0 = np.ascontiguousarray(midT[:16].T)  # [N,16]
    mid1 = np.ascontiguousarray(midT[16:].T).reshape(n, 16, 3)
    conv0 = (mid0 @ np.asarray(lin2_w0, f)) / 4.0 * a
    conv1 = np.einsum("nuc,uw->nwc", mid1, np.asarray(lin2_w1, f)) / 4.0 * a[:, :, None]
    conv = np.concatenate([conv0, conv1.reshape(n, 24)], axis=1)
    ang = 0.1 * (mid0 @ np.asarray(lin3_w, f)) / 4.0 * a
    mask = np.concatenate([np.ones(40, f), np.zeros(24, f)])
    sin = 1.0 - mask + np.sin(ang) * mask
    y = np.cos(ang) * sc + sin * conv
    sig = _sigmoid(y[:, :32])
    h0 = y[:, :32] * sig
    gates = _sigmoid(y[:, 32:40])
    h1 = y[:, 40:].reshape(n, 8, 3) * gates[:, :, None]

    # ---- layer 2 (host except w2) ----
    inv32, inv8, inv40 = f(1 / np.sqrt(32.0)), f(1 / np.sqrt(8.0)), f(1 / np.sqrt(40.0))
    sc2 = (h0 @ np.asarray(sc2_w, f)) * inv32 * a
    y0 = (h0 @ np.asarray(lin1b_w0, f)) * inv32 * a
    y1 = np.einsum("nuc,uw->nwc", h1, np.asarray(lin1b_w1, f)) * inv8 * a[:, :, None]
    xs0T = y0.T[:, src_s]                    # [32,E]
    xs1T = y1.reshape(n, 24).T[:, src_s]     # [24,E] rows u*3+c
    # precompute all w2-independent factors before blocking on the fetch
    xs0T *= sh0                              # xs0*sh0
    d = (xs1T.reshape(8, 3, E) * sh1T[None, :, :]).sum(axis=1)  # [8,E]
    d *= f(1.0 / S3)
    efbT = np.zeros((40, E), f)              # pre-touch pages before the join
    w2T = run.w2T()       # [40,E] layer-2 weights (blocks on device fetch)
    np.multiply(w2T[:32], xs0T, out=efbT[:32])
    np.multiply(w2T[32:], d, out=efbT[32:])
    mid2 = segsumT(efbT).T * inv_nn          # [N,40]
    conv2 = (mid2 @ np.asarray(lin2b_w, f)) * inv40 * a
    ang2 = 0.1 * (mid2 @ np.asarray(lin3b_w, f)) * inv40 * a
    return (np.cos(ang2) * sc2 + np.sin(ang2) * conv2).astype(np.float32)


def _warmup():
    """Initialize the device stack at import: bass build, jit trace, NEFF
    compile/load, device init and one dummy execution. Shapes (not values)
    key all caches, so kernel() later only pays transfers + execution."""
    try:
        es_dev, _ = _put_es_sharded(np.zeros((E, 16), np.float32))
        if es_dev is None:
            es_dev = np.zeros((N_CORES * 16, E_PAD), np.float16)
        ins = {
            "es_t": es_dev,
            "w1c": np.zeros((N_CORES * 16, 128), np.float16),
            "w2bd": np.zeros((N_CORES * 128, 72), np.float16),
        }
        arrs = _launch_spmd(ins)
        for v in arrs.values():
            np.asarray(v)
    except Exception:
        _CACHED.pop("jit", None)


_warmup()
